# revision 17
# baseline (speedup 1.0000x reference)
"""Trainium2 Bass kernel for nn_ARPrior (stacked causal-prior MLPs).

Network (per sample, latent D=32, L=31 stacked layers):
    zin = z[:, :31]
    h1[l] = relu(W1m[l] @ zin + b1[l])   # [128], W1m causally masked
    h2[l] = relu(W2[l]  @ h1[l] + b2[l]) # [64]
    out[l] = Wout[l] @ h2[l] + bout[l]   # [2]  (mu, logvar)
    mus = [bout0[0], out[:,0]]; lvs = [bout0[1], out[:,1]]

Mapping (pure data parallel, batch 65536 sharded 8 ways -> 8192/core):
  - L1: K=31 -> 4 layers packed in the 128x128 PE array via row tiling
    (tile_position=(32i,0)), each writing its own PSUM bank.
  - L2: K=128, M=64 -> 2 layers packed via column tiling
    (tile_position=(0,0)/(0,64)) into one PSUM bank.
  - L3: M=2 per layer -> all 31 layers' output weights embedded in a
    block-diagonal [128,64] stationary per layer-pair, accumulated over
    16 matmuls into a single [64,512] PSUM tile per batch tile. Output
    columns are pre-arranged [mu(32) | logvar(32)]; bias adds bout/bout0.
  - Batch tiles are processed in blocks of 2; within a block each
    stationary's matmuls are emitted adjacently so walrus --enable-ldw-opt
    can drop the duplicate LDWEIGHTS (weight reload dominates PE time
    otherwise).
  - PSUM drain (bias+relu+fp16 cast) is split greedily between ScalarE
    (activation Relu, ~2x rate with fp16 output) and VectorE
    (tensor_scalar add+max).

Host does all weight masking/stacking/transposing; device output is
[64, 8192] f32 per core, host concatenates and transposes.
"""

import sys

if "/opt/trn_rl_repo" not in sys.path:
    sys.path.insert(0, "/opt/trn_rl_repo")

import numpy as np

B = 65536
D = 32
L = 31
NCORES = 8
BC = B // NCORES  # 8192 per-core batch
FD = 1024           # L1 drain tile free dim (2 layers x 512 batch)
NT = BC // (FD // 2)  # 16 batch tiles of 512 per core

# layer pairs for L2/L3; last pair duplicates layer 30 (its W3 block is zero)
PAIRS = [(2 * q, min(2 * q + 1, L - 1)) for q in range(16)]

F16 = np.float16

DEDUP_LDW = True  # delete redundant InstLdweights post-schedule

_NC_CACHE = {}
LAST_RESULT = None  # BassKernelResults of the most recent run (for test.py)


def _dedup_ldweights(nc):
    """Remove LDWEIGHTS that reload the exact weights already resident in the
    same PE-array region. Runs after Tile scheduling (instruction order and
    semaphores final) and before Bacc lowering. Conservative: any overlapping
    region load or tiling-mode change invalidates, and only sync-free
    duplicates are deleted.
    """
    import concourse.mybir as mybir

    PE = mybir.EngineType.PE
    removed = 0
    for bb in nc.m.functions[0].blocks:
        loaded = {}
        cur_mode = None
        todel = []
        for ins in bb.instructions:
            if getattr(ins, "engine", None) != PE:
                continue
            tn = type(ins).__name__
            if tn == "InstLdweights":
                tp = ins.tile_position or (0, 0)
                tsz = ins.tile_size or (128, 128)
                if tsz != cur_mode:
                    loaded.clear()
                    cur_mode = tsz
                region = (tp[0], tp[0] + tsz[0], tp[1], tp[1] + tsz[1])
                ap = ins.ins[0]
                sig = (
                    getattr(ap, "memref", None),
                    getattr(ap, "offset", None),
                    str(getattr(ap, "ap", None)),
                    str(getattr(ap, "dtype", None)),
                    tuple(tp),
                    tuple(tsz),
                )
                si = ins.sync_info
                clean = si is None or (not si.on_wait and not si.on_update)
                if loaded.get(region) == sig and clean:
                    todel.append(ins)
                    removed += 1
                    continue
                for rk in list(loaded):
                    if not (
                        rk[1] <= region[0]
                        or region[1] <= rk[0]
                        or rk[3] <= region[2]
                        or region[3] <= rk[2]
                    ):
                        del loaded[rk]
                loaded[region] = sig
            elif tn == "InstMatmult":
                tsz = ins.tile_size or (128, 128)
                if tuple(tsz) != (cur_mode and tuple(cur_mode)):
                    if tsz != cur_mode:
                        loaded.clear()
                        cur_mode = tsz
        for ins in todel:
            bb.instructions.remove(ins)
            nc.inst_map.pop(ins.name, None)
    return removed


def _build_nc():
    import concourse.mybir as mybir
    from concourse import bacc, tile

    f32 = mybir.dt.float32
    f16 = mybir.dt.float16
    ADD = mybir.AluOpType.add
    MAX = mybir.AluOpType.max
    RELU = mybir.ActivationFunctionType.Relu

    nc = bacc.Bacc("TRN2", target_bir_lowering=False, debug=False)

    zT4_d = nc.declare_dram_parameter("zT4", [128, BC], f16, isOutput=False)
    w1_d = nc.declare_dram_parameter("w1s", [128, 16 * 128], f16, isOutput=False)
    w2_d = nc.declare_dram_parameter("w2s", [128, 16 * 128], f16, isOutput=False)
    w3_d = nc.declare_dram_parameter("w3s", [128, 16 * 64], f16, isOutput=False)
    b2_d = nc.declare_dram_parameter("b2s", [128, 16], f32, isOutput=False)
    b3_d = nc.declare_dram_parameter("b3s", [128, 1], f32, isOutput=False)
    out_d = nc.declare_dram_parameter("out", [64, BC], f32, isOutput=True)

    # greedy DVE/ACT balance for PSUM drains (calibrated ns per op at FD)
    eng_time = [0.0, 0.0]

    def dve_cost(fd):
        return (fd + 256) / 0.96

    def act_cost(fd):
        return (fd + 308) / 1.2

    HF = FD // 2  # single-matmul moving dim (PSUM bank limit)

    with tile.TileContext(nc) as tc:
        with (
            tc.tile_pool(name="const", bufs=1) as const,
            tc.tile_pool(name="l1ps", bufs=4, space="PSUM") as l1ps,
            tc.tile_pool(name="l2ps", bufs=2, space="PSUM") as l2ps,
            tc.tile_pool(name="l3ps", bufs=2, space="PSUM") as l3ps,
            tc.tile_pool(name="h1p", bufs=10) as h1p,
            tc.tile_pool(name="h2p", bufs=18) as h2p,
            tc.tile_pool(name="outp", bufs=3) as outp,
        ):
            zt_all = const.tile([128, BC], f16, name="zt_all")
            nc.sync.dma_start(zt_all[:], zT4_d[:, :])
            w1t = const.tile([128, 16 * 128], f16, name="w1t")
            nc.sync.dma_start(w1t[:], w1_d[:, :])
            w2t = const.tile([128, 16 * 128], f16, name="w2t")
            nc.sync.dma_start(w2t[:], w2_d[:, :])
            w3t = const.tile([128, 16 * 64], f16, name="w3t")
            nc.sync.dma_start(w3t[:], w3_d[:, :])
            b2t = const.tile([128, 16], f32, name="b2t")
            nc.sync.dma_start(b2t[:], b2_d[:, :])
            b3t = const.tile([128, 1], f32, name="b3t")
            nc.sync.dma_start(b3t[:], b3_d[:, :])

            def drain(dst, src, bias_ap, relu=True):
                fd = src.shape[-1]
                if eng_time[0] + dve_cost(fd) <= eng_time[1] + act_cost(fd):
                    eng_time[0] += dve_cost(fd)
                    if relu and bias_ap is None:
                        nc.vector.tensor_scalar(dst, src, 0.0, None, MAX)
                    elif relu:
                        nc.vector.tensor_scalar(dst, src, bias_ap, 0.0, ADD, MAX)
                    else:
                        nc.vector.tensor_scalar(dst, src, bias_ap, None, ADD)
                else:
                    eng_time[1] += act_cost(fd)
                    fn = RELU if relu else mybir.ActivationFunctionType.Identity
                    nc.scalar.activation(
                        dst, src, fn, bias=0.0 if bias_ap is None else bias_ap
                    )

            for t in range(NT):
                zt = zt_all[:, HF * t : HF * (t + 1)]
                h2_tiles = []
                for g in range(8):
                    lyrs = [x for x in range(4 * g, 4 * g + 4) if x < L]
                    h1_tiles = []
                    for i, lyr in enumerate(lyrs):
                        p, k = divmod(lyr, 2)  # w1s pass block / slot in block
                        ro = 64 * (p % 2) + 32 * k
                        ps = l1ps.tile(
                            [128, HF], f32, tag="l1", name=f"ps1_{t}_{lyr}"
                        )
                        nc.tensor.matmul(
                            ps[:],
                            lhsT=w1t[ro : ro + 32, 128 * p : 128 * (p + 1)],
                            rhs=zt[ro : ro + 32, :],
                            start=True,
                            stop=True,
                            tile_position=(ro, 0),
                        )
                        h1 = h1p.tile(
                            [128, HF], f16, tag="h1", name=f"h1_{t}_{lyr}"
                        )
                        drain(h1[:], ps[:], None)
                        h1_tiles.append(h1)
                    if len(lyrs) == 3:
                        h1_tiles.append(h1_tiles[2])
                    for j in range(2):
                        q = 2 * g + j
                        ha = h1_tiles[2 * j]
                        hb = h1_tiles[2 * j + 1]
                        ps2 = l2ps.tile(
                            [128, HF], f32, tag="l2", name=f"ps2_{t}_{q}"
                        )
                        nc.tensor.matmul(
                            ps2[0:64, :],
                            lhsT=w2t[:, 128 * q : 128 * q + 64],
                            rhs=ha[:],
                            start=True,
                            stop=True,
                            tile_position=(0, 0),
                        )
                        if q < 15:
                            nc.tensor.matmul(
                                ps2[64:128, :],
                                lhsT=w2t[:, 128 * q + 64 : 128 * (q + 1)],
                                rhs=hb[:],
                                start=True,
                                stop=True,
                                tile_position=(0, 64),
                            )
                        h2 = h2p.tile(
                            [128, HF], f16, tag="h2", name=f"h2_{t}_{q}"
                        )
                        drain(h2[:], ps2[:], b2t[:, q : q + 1])
                        h2_tiles.append(h2)
                ps3 = l3ps.tile([64, HF], f32, tag="l3", name=f"ps3_{t}")
                for q in range(16):
                    nc.tensor.matmul(
                        ps3[:],
                        lhsT=w3t[:, 64 * q : 64 * (q + 1)],
                        rhs=h2_tiles[q][:],
                        start=(q == 0),
                        stop=(q == 15),
                    )
                osb = outp.tile([64, HF], f32, tag="o", name=f"osb_{t}")
                drain(osb[:], ps3[:], b3t[0:64, 0:1], relu=False)
                nc.sync.dma_start(out_d[:, HF * t : HF * (t + 1)], osb[:])

    if DEDUP_LDW:
        n = _dedup_ldweights(nc)
        print(f"dedup_ldweights removed {n}")
    nc.finalize()
    return nc


def _get_nc():
    if "nc" not in _NC_CACHE:
        _NC_CACHE["nc"] = _build_nc()
    return _NC_CACHE["nc"]


def _prep_shared(W1, b1, W2, b2, Wout, bout, bout0):
    W1 = np.asarray(W1, np.float32)
    b1 = np.asarray(b1, np.float32)
    W2 = np.asarray(W2, np.float32)
    b2 = np.asarray(b2, np.float32)
    Wout = np.asarray(Wout, np.float32)
    bout = np.asarray(bout, np.float32)
    bout0 = np.asarray(bout0, np.float32)

    mask = np.tril(np.ones((L, L), np.float32))
    W1m = W1 * mask[:, None, :]  # [31, 128, 31]

    # L1 stationaries: pass p holds layers (2p, 2p+1) at row offset 64*(p%2)
    # so consecutive passes use disjoint PE row groups (LDWEIGHTS overlap).
    # Row 31 of each 32-row block is the layer bias (K=32 ones-row trick).
    w1s = np.zeros((128, 16 * 128), F16)
    for p in range(16):
        off = 64 * (p % 2)
        for k, lyr in enumerate([x for x in (2 * p, 2 * p + 1) if x < L]):
            ro = off + 32 * k
            w1s[ro : ro + L, 128 * p : 128 * (p + 1)] = W1m[lyr].T.astype(F16)
            w1s[ro + L, 128 * p : 128 * (p + 1)] = b1[lyr].astype(F16)

    w2s = np.zeros((128, 16 * 128), F16)
    b2s = np.zeros((128, 16), np.float32)
    for q, (a, bb) in enumerate(PAIRS):
        w2s[:, 128 * q : 128 * q + 64] = W2[a].T.astype(F16)
        w2s[:, 128 * q + 64 : 128 * (q + 1)] = W2[bb].T.astype(F16)
        b2s[0:64, q] = b2[a]
        b2s[64:128, q] = b2[bb]

    w3s = np.zeros((128, 16 * 64), F16)
    for q, (a, bb) in enumerate(PAIRS):
        blk = np.zeros((128, 64), np.float32)
        blk[0:64, 1 + a] = Wout[a][0]
        blk[0:64, 33 + a] = Wout[a][1]
        if 2 * q + 1 <= L - 1:  # real second layer (not the dup)
            blk[64:128, 1 + bb] = Wout[bb][0]
            blk[64:128, 33 + bb] = Wout[bb][1]
        w3s[:, 64 * q : 64 * (q + 1)] = blk.astype(F16)

    b3h = np.zeros(64, np.float32)
    b3h[0] = bout0[0]
    b3h[1 : 1 + L] = bout[:, 0]
    b3h[32] = bout0[1]
    b3h[33 : 33 + L] = bout[:, 1]
    b3s = np.concatenate([b3h, b3h]).reshape(128, 1)  # both L3 psum halves

    return w1s, w2s, w3s, b2s, b3s


def kernel(z, W1, b1, W2, b2, Wout, bout, bout0):
    global LAST_RESULT
    from concourse.bass_utils import run_bass_kernel_spmd

    z = np.asarray(z, np.float32)
    w1s, w2s, w3s, b2s, b3s = _prep_shared(W1, b1, W2, b2, Wout, bout, bout0)

    zin_T = np.ascontiguousarray(z[:, :L].T).astype(F16)  # [31, 65536]
    in_maps = []
    for c in range(NCORES):
        sl = zin_T[:, BC * c : BC * (c + 1)]
        zt4 = np.zeros((128, BC), F16)
        for i in range(4):
            zt4[32 * i : 32 * i + L] = sl
            zt4[32 * i + L] = 1.0  # ones row: folds b1 via the K=32 bias row
        in_maps.append(
            {
                "zT4": zt4,
                "w1s": w1s,
                "w2s": w2s,
                "w3s": w3s,
                "b2s": b2s,
                "b3s": b3s,
            }
        )

    nc = _get_nc()
    res = run_bass_kernel_spmd(nc, in_maps, core_ids=list(range(NCORES)))
    LAST_RESULT = res

    big = np.concatenate([res.results[c]["out"] for c in range(NCORES)], axis=1)
    mus = np.ascontiguousarray(big[:32].T).astype(np.float32, copy=False)
    lvs = np.ascontiguousarray(big[32:].T).astype(np.float32, copy=False)
    return mus, lvs


# revision 18
# speedup vs baseline: 1.2389x; 1.2389x over previous
"""Trainium2 Bass kernel for nn_ARPrior (stacked causal-prior MLPs).

Network (per sample, latent D=32, L=31 stacked layers):
    zin = z[:, :31]
    h1[l] = relu(W1m[l] @ zin + b1[l])   # [128], W1m causally masked
    h2[l] = relu(W2[l]  @ h1[l] + b2[l]) # [64]
    out[l] = Wout[l] @ h2[l] + bout[l]   # [2]  (mu, logvar)
    mus = [bout0[0], out[:,0]]; lvs = [bout0[1], out[:,1]]

Mapping (pure data parallel, batch 65536 sharded 8 ways -> 8192/core):
  - L1: K=31 -> 4 layers packed in the 128x128 PE array via row tiling
    (tile_position=(32i,0)), each writing its own PSUM bank.
  - L2: K=128, M=64 -> 2 layers packed via column tiling
    (tile_position=(0,0)/(0,64)) into one PSUM bank.
  - L3: M=2 per layer -> all 31 layers' output weights embedded in a
    block-diagonal [128,64] stationary per layer-pair, accumulated over
    16 matmuls into a single [64,512] PSUM tile per batch tile. Output
    columns are pre-arranged [mu(32) | logvar(32)]; bias adds bout/bout0.
  - Batch tiles are processed in blocks of 2; within a block each
    stationary's matmuls are emitted adjacently so walrus --enable-ldw-opt
    can drop the duplicate LDWEIGHTS (weight reload dominates PE time
    otherwise).
  - PSUM drain (bias+relu+fp16 cast) is split greedily between ScalarE
    (activation Relu, ~2x rate with fp16 output) and VectorE
    (tensor_scalar add+max).

Host does all weight masking/stacking/transposing; device output is
[64, 8192] f32 per core, host concatenates and transposes.
"""

import sys

if "/opt/trn_rl_repo" not in sys.path:
    sys.path.insert(0, "/opt/trn_rl_repo")

import numpy as np

B = 65536
D = 32
L = 31
NCORES = 8
BC = B // NCORES  # 8192 per-core batch
FD = 1024           # L1 drain tile free dim (2 layers x 512 batch)
NT = BC // (FD // 2)  # 16 batch tiles of 512 per core

# layer pairs for L2/L3; last pair duplicates layer 30 (its W3 block is zero)
PAIRS = [(2 * q, min(2 * q + 1, L - 1)) for q in range(16)]

F16 = np.float16

DEDUP_LDW = False  # delete redundant InstLdweights post-schedule

_NC_CACHE = {}
LAST_RESULT = None  # BassKernelResults of the most recent run (for test.py)


def _dedup_ldweights(nc):
    """Remove LDWEIGHTS that reload the exact weights already resident in the
    same PE-array region. Runs after Tile scheduling (instruction order and
    semaphores final) and before Bacc lowering. Conservative: any overlapping
    region load or tiling-mode change invalidates, and only sync-free
    duplicates are deleted.
    """
    import concourse.mybir as mybir

    PE = mybir.EngineType.PE
    removed = 0
    for bb in nc.m.functions[0].blocks:
        loaded = {}
        cur_mode = None
        todel = []
        for ins in bb.instructions:
            if getattr(ins, "engine", None) != PE:
                continue
            tn = type(ins).__name__
            if tn == "InstLdweights":
                tp = ins.tile_position or (0, 0)
                tsz = ins.tile_size or (128, 128)
                if tsz != cur_mode:
                    loaded.clear()
                    cur_mode = tsz
                region = (tp[0], tp[0] + tsz[0], tp[1], tp[1] + tsz[1])
                ap = ins.ins[0]
                sig = (
                    getattr(ap, "memref", None),
                    getattr(ap, "offset", None),
                    str(getattr(ap, "ap", None)),
                    str(getattr(ap, "dtype", None)),
                    tuple(tp),
                    tuple(tsz),
                )
                si = ins.sync_info
                clean = si is None or (not si.on_wait and not si.on_update)
                if loaded.get(region) == sig and clean:
                    todel.append(ins)
                    removed += 1
                    continue
                for rk in list(loaded):
                    if not (
                        rk[1] <= region[0]
                        or region[1] <= rk[0]
                        or rk[3] <= region[2]
                        or region[3] <= rk[2]
                    ):
                        del loaded[rk]
                loaded[region] = sig
            elif tn == "InstMatmult":
                tsz = ins.tile_size or (128, 128)
                if tuple(tsz) != (cur_mode and tuple(cur_mode)):
                    if tsz != cur_mode:
                        loaded.clear()
                        cur_mode = tsz
        for ins in todel:
            bb.instructions.remove(ins)
            nc.inst_map.pop(ins.name, None)
    return removed


def _build_nc():
    import concourse.mybir as mybir
    from concourse import bacc, tile

    f32 = mybir.dt.float32
    f16 = mybir.dt.float16
    ADD = mybir.AluOpType.add
    MAX = mybir.AluOpType.max
    RELU = mybir.ActivationFunctionType.Relu

    nc = bacc.Bacc("TRN2", target_bir_lowering=False, debug=False)

    zT4_d = nc.declare_dram_parameter("zT4", [128, BC], f16, isOutput=False)
    w1_d = nc.declare_dram_parameter("w1s", [128, 16 * 128], f16, isOutput=False)
    w2_d = nc.declare_dram_parameter("w2s", [128, 16 * 128], f16, isOutput=False)
    w3_d = nc.declare_dram_parameter("w3s", [128, 16 * 64], f16, isOutput=False)
    b1_d = nc.declare_dram_parameter("b1s", [128, L], f32, isOutput=False)
    b2_d = nc.declare_dram_parameter("b2s", [128, 16], f32, isOutput=False)
    b3_d = nc.declare_dram_parameter("b3s", [128, 1], f32, isOutput=False)
    out_d = nc.declare_dram_parameter("out", [64, BC], f32, isOutput=True)

    # greedy DVE/ACT balance for PSUM drains (calibrated ns per op at FD)
    eng_time = [0.0, 0.0]

    def dve_cost(fd):
        return 658.0

    def act_cost(fd):
        return 570.0

    HF = FD // 2  # single-matmul moving dim (PSUM bank limit)

    with tile.TileContext(nc) as tc:
        with (
            tc.tile_pool(name="const", bufs=1) as const,
            tc.tile_pool(name="l1ps", bufs=4, space="PSUM") as l1ps,
            tc.tile_pool(name="l2ps", bufs=2, space="PSUM") as l2ps,
            tc.tile_pool(name="l3ps", bufs=2, space="PSUM") as l3ps,
            tc.tile_pool(name="h1p", bufs=10) as h1p,
            tc.tile_pool(name="h2p", bufs=18) as h2p,
            tc.tile_pool(name="outp", bufs=3) as outp,
        ):
            zt_all = const.tile([128, BC], f16, name="zt_all")
            nc.sync.dma_start(zt_all[:], zT4_d[:, :])
            w1t = const.tile([128, 16 * 128], f16, name="w1t")
            nc.sync.dma_start(w1t[:], w1_d[:, :])
            w2t = const.tile([128, 16 * 128], f16, name="w2t")
            nc.sync.dma_start(w2t[:], w2_d[:, :])
            w3t = const.tile([128, 16 * 64], f16, name="w3t")
            nc.sync.dma_start(w3t[:], w3_d[:, :])
            b1t = const.tile([128, L], f32, name="b1t")
            nc.sync.dma_start(b1t[:], b1_d[:, :])
            b2t = const.tile([128, 16], f32, name="b2t")
            nc.sync.dma_start(b2t[:], b2_d[:, :])
            b3t = const.tile([128, 1], f32, name="b3t")
            nc.sync.dma_start(b3t[:], b3_d[:, :])

            def drain(dst, src, bias_ap, relu=True):
                fd = src.shape[-1]
                if eng_time[0] + dve_cost(fd) <= eng_time[1] + act_cost(fd):
                    eng_time[0] += dve_cost(fd)
                    if relu and bias_ap is None:
                        nc.vector.tensor_scalar(dst, src, 0.0, None, MAX)
                    elif relu:
                        nc.vector.tensor_scalar(dst, src, bias_ap, 0.0, ADD, MAX)
                    else:
                        nc.vector.tensor_scalar(dst, src, bias_ap, None, ADD)
                else:
                    eng_time[1] += act_cost(fd)
                    fn = RELU if relu else mybir.ActivationFunctionType.Identity
                    nc.scalar.activation(
                        dst, src, fn, bias=0.0 if bias_ap is None else bias_ap
                    )

            for t in range(NT):
                zt = zt_all[:, HF * t : HF * (t + 1)]
                h2_tiles = []
                for g in range(8):
                    lyrs = [x for x in range(4 * g, 4 * g + 4) if x < L]
                    h1_tiles = []
                    for i, lyr in enumerate(lyrs):
                        ro = 32 * i
                        ps = l1ps.tile(
                            [128, HF], f32, tag="l1", name=f"ps1_{t}_{lyr}"
                        )
                        nc.tensor.matmul(
                            ps[:],
                            lhsT=w1t[ro : ro + L, 128 * g : 128 * (g + 1)],
                            rhs=zt[ro : ro + L, :],
                            start=True,
                            stop=True,
                            tile_position=(ro, 0),
                        )
                        h1 = h1p.tile(
                            [128, HF], f16, tag="h1", name=f"h1_{t}_{lyr}"
                        )
                        drain(h1[:], ps[:], b1t[:, lyr : lyr + 1])
                        h1_tiles.append(h1)
                    if len(lyrs) == 3:
                        h1_tiles.append(h1_tiles[2])
                    for j in range(2):
                        q = 2 * g + j
                        ha = h1_tiles[2 * j]
                        hb = h1_tiles[2 * j + 1]
                        ps2 = l2ps.tile(
                            [128, HF], f32, tag="l2", name=f"ps2_{t}_{q}"
                        )
                        nc.tensor.matmul(
                            ps2[0:64, :],
                            lhsT=w2t[:, 128 * q : 128 * q + 64],
                            rhs=ha[:],
                            start=True,
                            stop=True,
                            tile_position=(0, 0),
                        )
                        nc.tensor.matmul(
                            ps2[64:128, :],
                            lhsT=w2t[:, 128 * q + 64 : 128 * (q + 1)],
                            rhs=hb[:],
                            start=True,
                            stop=True,
                            tile_position=(0, 64),
                        )
                        h2 = h2p.tile(
                            [128, HF], f16, tag="h2", name=f"h2_{t}_{q}"
                        )
                        drain(h2[:], ps2[:], b2t[:, q : q + 1])
                        h2_tiles.append(h2)
                ps3 = l3ps.tile([64, HF], f32, tag="l3", name=f"ps3_{t}")
                for q in range(16):
                    nc.tensor.matmul(
                        ps3[:],
                        lhsT=w3t[:, 64 * q : 64 * (q + 1)],
                        rhs=h2_tiles[q][:],
                        start=(q == 0),
                        stop=(q == 15),
                    )
                osb = outp.tile([64, HF], f32, tag="o", name=f"osb_{t}")
                drain(osb[:], ps3[:], b3t[0:64, 0:1], relu=False)
                nc.sync.dma_start(out_d[:, HF * t : HF * (t + 1)], osb[:])

    if DEDUP_LDW:
        n = _dedup_ldweights(nc)
        print(f"dedup_ldweights removed {n}")
    nc.finalize()
    return nc


def _get_nc():
    if "nc" not in _NC_CACHE:
        _NC_CACHE["nc"] = _build_nc()
    return _NC_CACHE["nc"]


def _prep_shared(W1, b1, W2, b2, Wout, bout, bout0):
    W1 = np.asarray(W1, np.float32)
    b1 = np.asarray(b1, np.float32)
    W2 = np.asarray(W2, np.float32)
    b2 = np.asarray(b2, np.float32)
    Wout = np.asarray(Wout, np.float32)
    bout = np.asarray(bout, np.float32)
    bout0 = np.asarray(bout0, np.float32)

    mask = np.tril(np.ones((L, L), np.float32))
    W1m = W1 * mask[:, None, :]  # [31, 128, 31]

    w1s = np.zeros((128, 16 * 128), F16)
    for g in range(8):
        for i in range(4):
            lyr = 4 * g + i
            if lyr >= L:
                break
            w1s[32 * i : 32 * i + L, 128 * g : 128 * (g + 1)] = W1m[lyr].T.astype(
                F16
            )
    b1s = np.ascontiguousarray(b1.T)  # [128, 31]

    w2s = np.zeros((128, 16 * 128), F16)
    b2s = np.zeros((128, 16), np.float32)
    for q, (a, bb) in enumerate(PAIRS):
        w2s[:, 128 * q : 128 * q + 64] = W2[a].T.astype(F16)
        w2s[:, 128 * q + 64 : 128 * (q + 1)] = W2[bb].T.astype(F16)
        b2s[0:64, q] = b2[a]
        b2s[64:128, q] = b2[bb]

    w3s = np.zeros((128, 16 * 64), F16)
    for q, (a, bb) in enumerate(PAIRS):
        blk = np.zeros((128, 64), np.float32)
        blk[0:64, 1 + a] = Wout[a][0]
        blk[0:64, 33 + a] = Wout[a][1]
        if 2 * q + 1 <= L - 1:  # real second layer (not the dup)
            blk[64:128, 1 + bb] = Wout[bb][0]
            blk[64:128, 33 + bb] = Wout[bb][1]
        w3s[:, 64 * q : 64 * (q + 1)] = blk.astype(F16)

    b3h = np.zeros(64, np.float32)
    b3h[0] = bout0[0]
    b3h[1 : 1 + L] = bout[:, 0]
    b3h[32] = bout0[1]
    b3h[33 : 33 + L] = bout[:, 1]
    b3s = np.concatenate([b3h, b3h]).reshape(128, 1)  # both L3 psum halves

    return w1s, w2s, w3s, b1s, b2s, b3s


def kernel(z, W1, b1, W2, b2, Wout, bout, bout0):
    global LAST_RESULT
    from concourse.bass_utils import run_bass_kernel_spmd

    z = np.asarray(z, np.float32)
    w1s, w2s, w3s, b1s, b2s, b3s = _prep_shared(W1, b1, W2, b2, Wout, bout, bout0)

    zin_T = np.ascontiguousarray(z[:, :L].T).astype(F16)  # [31, 65536]
    in_maps = []
    for c in range(NCORES):
        sl = zin_T[:, BC * c : BC * (c + 1)]
        zt4 = np.zeros((128, BC), F16)
        for i in range(4):
            zt4[32 * i : 32 * i + L] = sl
        in_maps.append(
            {
                "zT4": zt4,
                "w1s": w1s,
                "w2s": w2s,
                "w3s": w3s,
                "b1s": b1s,
                "b2s": b2s,
                "b3s": b3s,
            }
        )

    nc = _get_nc()
    res = run_bass_kernel_spmd(nc, in_maps, core_ids=list(range(NCORES)))
    LAST_RESULT = res

    big = np.concatenate([res.results[c]["out"] for c in range(NCORES)], axis=1)
    mus = np.ascontiguousarray(big[:32].T).astype(np.float32, copy=False)
    lvs = np.ascontiguousarray(big[32:].T).astype(np.float32, copy=False)
    return mus, lvs
